# revision 3
# baseline (speedup 1.0000x reference)
"""L21 norm kernel for Trainium2 (Bass/Tile), 8-core SPMD.

Computes sum_j sqrt(sum_i S[i,j]^2) for S of shape [8192, 16384] fp32.

Sharding: S is split along columns into 8 shards of [8192, 2048] (one per
NeuronCore). Each core computes the sum of its columns' L2 norms as a
scalar; the host sums the 8 partial scalars.

Per-core dataflow (memory-bound; 64 MiB HBM read per core, ~187 us floor
at 358 GB/s per-NC):
  - 32 tiles of [128 partitions, 2 rows, 2048 cols] fp32 (2 MiB HWDGE
    DMAs; each partition's slice is 16 KiB contiguous in DRAM).
  - ACT engine: square with bf16 output (also the dtype cast for PE).
  - Partition-axis reduction is split so neither engine paces the DMA
    stream: per tile, row-slice q=0 goes to PE (ones[128,1]^T @ sq
    matmuls accumulating into PSUM [1,2048] fp32) and row-slice q=1 is
    accumulated on DVE into a bf16 [128,2048] accumulator (2x mode).
    The DVE accumulator is folded into PSUM via 4 matmuls near the end;
    the last tile sends both row-slices to PE to keep the tail short.
  - Epilogue: ACT sqrt (PSUM -> SBUF), DVE free-axis reduce_sum -> [1,1],
    DMA to DRAM.
"""

import numpy as np

# Full problem shape (hardcoded per the harness contract).
R = 8192          # rows
C_FULL = 16384    # columns
N_CORES = 8
C = C_FULL // N_CORES  # 2048 columns per core
P = 128           # SBUF partitions
Q = 2             # rows per partition per tile (16 KiB contiguous DRAM)
T = R // (P * Q)  # tiles per core (32)
NBLK = 512        # matmul moving free dim (one PSUM bank of fp32)

_cached = None


def _build():
    """Build + schedule the per-core Bass program. Returns the Bacc object."""
    import concourse.bacc as bacc
    import concourse.tile as tile
    from concourse import mybir

    nc = bacc.Bacc(
        "TRN2",
        target_bir_lowering=False,
        debug=False,
        enable_asserts=False,
        num_devices=N_CORES,
    )

    s_dram = nc.dram_tensor("S", [R, C], mybir.dt.float32, kind="ExternalInput")
    out_dram = nc.dram_tensor("out", [1, 1], mybir.dt.float32, kind="ExternalOutput")

    s_ap = s_dram.ap()
    out_ap = out_dram.ap()

    # [T, P, Q, C]: tile t covers rows [t*P*Q, (t+1)*P*Q); partition p holds
    # Q consecutive rows -> 16 KiB contiguous DRAM per (t, p) descriptor.
    # The last full tile's worth of rows is instead handled as two
    # [P, 1, C] sub-tiles to shorten the serial epilogue chain.
    TF = T - 1  # number of full tiles (0..TF-1)
    s_view = s_ap.rearrange("(t p q) c -> t p q c", p=P, q=Q)
    s_tail = s_ap[TF * P * Q :, :].rearrange("(s p) c -> s p c", p=P)  # [Q, P, C]

    with tile.TileContext(nc) as tc:
        with (
            tc.tile_pool(name="io", bufs=6) as io_pool,
            tc.tile_pool(name="sqp", bufs=3) as sq_pool,
            tc.tile_pool(name="const", bufs=1) as const_pool,
            tc.tile_pool(name="ps", bufs=1, space="PSUM") as ps_pool,
            tc.tile_pool(name="fin", bufs=1) as fin_pool,
        ):
            # First input DMA before any const setup so the Sync engine
            # starts streaming as early as possible.
            x0 = io_pool.tile([P, Q, C], mybir.dt.float32, tag="x")
            nc.sync.dma_start(out=x0, in_=s_view[0])

            ones = const_pool.tile([P, 1], mybir.dt.bfloat16)
            nc.vector.memset(ones, 1.0)

            # DVE-side accumulator for q=1 row-slices.
            acc = const_pool.tile([P, C], mybir.dt.bfloat16)

            # Per-column sum of squares (4 PSUM banks).
            colsq = ps_pool.tile([1, C], mybir.dt.float32)

            # Dummy sqrt: pulls the sqrt ACT-table load out of the tail.
            warm = const_pool.tile([1, 1], mybir.dt.float32)
            nc.scalar.sqrt(out=warm, in_=ones[0:1, :])

            def pe_reduce(src, first, last):
                for b in range(C // NBLK):
                    nc.tensor.matmul(
                        colsq[:, b * NBLK : (b + 1) * NBLK],
                        ones,
                        src[:, b * NBLK : (b + 1) * NBLK],
                        start=first,
                        stop=(last and b == C // NBLK - 1),
                    )

            for t in range(TF):
                if t == 0:
                    x_tile = x0
                else:
                    x_tile = io_pool.tile([P, Q, C], mybir.dt.float32, tag="x")
                    nc.sync.dma_start(out=x_tile, in_=s_view[t])

                sq = sq_pool.tile([P, Q, C], mybir.dt.bfloat16, tag="sq")
                nc.scalar.square(out=sq, in_=x_tile)

                # q=0 row-slice -> PE psum accumulate.
                pe_reduce(sq[:, 0, :], first=(t == 0), last=False)

                # q=1 row-slice -> DVE bf16 accumulator; the last full tile
                # goes to PE (the accumulator is already folded by then).
                if t == 0:
                    nc.vector.tensor_copy(acc, sq[:, 1, :])
                elif t < TF - 1:
                    nc.vector.tensor_add(acc, acc, sq[:, 1, :])
                else:
                    pe_reduce(sq[:, 1, :], first=False, last=False)

                # Fold the DVE accumulator into PSUM while later tiles'
                # DMAs are still in flight.
                if t == TF - 2:
                    pe_reduce(acc, first=False, last=False)

            # Tail: two small [P, 1, C] sub-tiles keep the post-last-byte
            # chain short (small square, 4 matmuls).
            for s in range(Q):
                xs_tile = io_pool.tile([P, 1, C], mybir.dt.float32, tag="xs")
                nc.sync.dma_start(out=xs_tile[:, 0, :], in_=s_tail[s])
                sqs = sq_pool.tile([P, 1, C], mybir.dt.bfloat16, tag="sqs")
                nc.scalar.square(out=sqs, in_=xs_tile)
                pe_reduce(sqs[:, 0, :], first=False, last=(s == Q - 1))

            # Per-block sqrt + partial reduce pipeline behind the last MMs.
            norms = fin_pool.tile([1, C], mybir.dt.float32)
            part = fin_pool.tile([1, C // NBLK], mybir.dt.float32)
            for b in range(C // NBLK):
                blk = slice(b * NBLK, (b + 1) * NBLK)
                nc.scalar.sqrt(out=norms[:, blk], in_=colsq[:, blk])
                nc.vector.reduce_sum(
                    out=part[:, b : b + 1], in_=norms[:, blk], axis=mybir.AxisListType.X
                )

            total = fin_pool.tile([1, 1], mybir.dt.float32)
            nc.vector.reduce_sum(out=total, in_=part, axis=mybir.AxisListType.X)

            nc.sync.dma_start(out=out_ap, in_=total)

    nc.compile()
    return nc


def _get_nc():
    global _cached
    if _cached is None:
        _cached = _build()
    return _cached


def _run(S: np.ndarray, trace: bool = False):
    from concourse import bass_utils

    assert S.shape == (R, C_FULL), S.shape
    S = np.ascontiguousarray(np.asarray(S, dtype=np.float32))

    nc = _get_nc()
    in_maps = [
        {"S": np.ascontiguousarray(S[:, i * C : (i + 1) * C])} for i in range(N_CORES)
    ]
    res = bass_utils.run_bass_kernel_spmd(
        nc, in_maps, core_ids=list(range(N_CORES)), trace=trace
    )
    partials = np.array(
        [res.results[i]["out"][0, 0] for i in range(N_CORES)], dtype=np.float64
    )
    out = np.float32(partials.sum())
    return out, res


def kernel(S: np.ndarray) -> np.ndarray:
    out, _ = _run(S, trace=False)
    return np.asarray(out, dtype=np.float32)


def run_traced(S: np.ndarray):
    """For test.py: returns (output, BassKernelResults) with NTFF trace."""
    return _run(S, trace=True)
